# revision 17
# baseline (speedup 1.0000x reference)
"""Additive-attention (tanh energy + softmax + context) kernel for 8 TRN2 NeuronCores.

Data-parallel over the batch dim B=128 -> 16 samples per core.

Software-pipelined per-core schedule over groups of 4 samples:
    E(g):   pre.T = W_h @ XposT + W_d @ XdynT (PE) -> tanh (ACT) -> energy
            columns (PE) -> es_g (DVE)
    SM(g):  batched group softmax: (e+b_a)*scale (DVE) -> exp (ACT) -> sums
            (DVE) -> totals/broadcast (PE ones-matmuls) -> 1/total, alpha (DVE)
    A(g):   alpha.T out (one PE transpose + one DMA per group)
    CTX(g): unnormalized ctx.T[:, b] += Xnat_chunk.T @ exp_col (PE, psum),
            then * (1/total), transpose, DMA out
  emitted as E(0), E(1), SM(0), E(2), SM(1), A(0), ..., then all CTX at the end,
  so the PE stream never blocks on the softmax DVE/ACT chain or on the
  (deprioritized) Xnat loads; xt/xd loads carry the phase-1 critical path.

Host side: shard B across 8 cores, cast X to bf16, pre-transpose layouts so every
device DMA is fully contiguous, run SPMD, gather outputs.
"""

import numpy as np
import ml_dtypes

B, T, H = 128, 2048, 128
N_CORES = 8
B_LOC = B // N_CORES      # 16 samples per core
TCH = 512                 # t-chunk for the pre matmuls (one psum bank)
NCH = T // TCH            # 4 chunks per sample
NJJ = T // 128            # 16 energy columns per sample
GRP = 4                   # samples per softmax/ctx group (= DMA batch)

_BF16 = ml_dtypes.bfloat16

_cache = {}


def _build_bass(n_samples=B_LOC):
    import concourse.bass as bass
    import concourse.tile as tile
    from concourse import bacc, mybir
    from contextlib import ExitStack

    f32 = mybir.dt.float32
    bf16 = mybir.dt.bfloat16
    AF = mybir.ActivationFunctionType
    OP = mybir.AluOpType

    assert n_samples % GRP == 0
    n_grp = n_samples // GRP

    nc = bacc.Bacc(
        "TRN2", target_bir_lowering=False, debug=False, num_devices=N_CORES
    )

    xposT = nc.dram_tensor("xposT", [n_grp, H, GRP * T], bf16, kind="ExternalInput").ap()
    xdynT = nc.dram_tensor("xdynT", [n_grp, H, GRP * T], bf16, kind="ExternalInput").ap()
    xposN = nc.dram_tensor("xposN", [n_grp, 128, GRP * T], bf16, kind="ExternalInput").ap()
    # scale_all[p, b, j] = (1 + softplus(beta) * acc_norm)[b, 128*j + p]
    scale_all = nc.dram_tensor("scale_all", [128, B_LOC, NJJ], f32, kind="ExternalInput").ap()
    whT = nc.dram_tensor("whT", [H, H], bf16, kind="ExternalInput").ap()
    wdT = nc.dram_tensor("wdT", [H, H], bf16, kind="ExternalInput").ap()
    wa = nc.dram_tensor("wa", [H, 1], bf16, kind="ExternalInput").ap()
    ident = nc.dram_tensor("ident", [128, 128], f32, kind="ExternalInput").ap()
    ones_col = nc.dram_tensor("ones_col", [128, 1], f32, kind="ExternalInput").ap()
    ones_row = nc.dram_tensor("ones_row", [1, 128], f32, kind="ExternalInput").ap()
    ba_col = nc.dram_tensor("ba_col", [128, 1], f32, kind="ExternalInput").ap()

    alpha_out = nc.dram_tensor("alpha_out", [B_LOC, T], f32, kind="ExternalOutput").ap()
    ctx_out = nc.dram_tensor("ctx_out", [B_LOC, H], f32, kind="ExternalOutput").ap()

    with tile.TileContext(nc) as tc, ExitStack() as ctx:
        consts = ctx.enter_context(tc.tile_pool(name="consts", bufs=1))
        xio_pool = ctx.enter_context(tc.tile_pool(name="xio", bufs=3))
        xn_pool = ctx.enter_context(tc.tile_pool(name="xnp", bufs=1))
        z_pool = ctx.enter_context(tc.tile_pool(name="z", bufs=4))
        sm_pool = ctx.enter_context(tc.tile_pool(name="sm", bufs=5))
        grp_pool = ctx.enter_context(tc.tile_pool(name="grp", bufs=4))
        ps_pre = ctx.enter_context(tc.tile_pool(name="pspre", bufs=4, space="PSUM"))
        ps_e = ctx.enter_context(tc.tile_pool(name="pse", bufs=2, space="PSUM"))
        ps_sm = ctx.enter_context(tc.tile_pool(name="pssm", bufs=1, space="PSUM"))
        ps_ctx = ctx.enter_context(tc.tile_pool(name="psctx", bufs=1, space="PSUM"))

        # constants on the scalar (ACT) HWDGE ring; bulk X loads on the sync ring
        whT_sb = consts.tile([H, H], bf16, tag="whT")
        nc.scalar.dma_start(whT_sb[:], whT[:])
        wdT_sb = consts.tile([H, H], bf16, tag="wdT")
        nc.scalar.dma_start(wdT_sb[:], wdT[:])
        wa_sb = consts.tile([H, 1], bf16, tag="wa")
        nc.scalar.dma_start(wa_sb[:], wa[:])
        ident_sb = consts.tile([128, 128], f32, tag="ident")
        nc.scalar.dma_start(ident_sb[:], ident[:])
        onesc_sb = consts.tile([128, 1], f32, tag="onesc")
        nc.scalar.dma_start(onesc_sb[:], ones_col[:])
        onesr_sb = consts.tile([1, 128], f32, tag="onesr")
        nc.scalar.dma_start(onesr_sb[:], ones_row[:])
        ba_sb = consts.tile([128, 1], f32, tag="ba")
        nc.scalar.dma_start(ba_sb[:], ba_col[:])
        sc_sb = consts.tile([128, B_LOC, NJJ], f32, tag="sc")
        nc.scalar.dma_start(sc_sb[:], scale_all[:])

        xn_sb = xn_pool.tile([128, B_LOC * T], bf16, tag="xn")

        # ---- input DMAs on the sync ring, phase-1-critical first ----
        xt_tiles = {}
        xd_tiles = {}

        def load_group(g):
            xt = xio_pool.tile([H, GRP * T], bf16, tag="xt", name=f"xt{g}")
            xd = xio_pool.tile([H, GRP * T], bf16, tag="xd", name=f"xd{g}")
            if g == 0:
                # split first group for a fast pipeline start
                hf = GRP * T // 2
                nc.sync.dma_start(xt[:, :hf], xposT[g][:, :hf])
                nc.sync.dma_start(xd[:, :hf], xdynT[g][:, :hf])
                nc.sync.dma_start(xt[:, hf:], xposT[g][:, hf:])
                nc.sync.dma_start(xd[:, hf:], xdynT[g][:, hf:])
            else:
                nc.sync.dma_start(xt[:], xposT[g])
                nc.sync.dma_start(xd[:], xdynT[g])
            xt_tiles[g] = xt
            xd_tiles[g] = xd

        def load_xn(g):
            nc.sync.dma_start(
                xn_sb[:, g * GRP * T:(g + 1) * GRP * T], xposN[g]
            )

        for g in range(n_grp):
            load_group(g)
        for g in range(n_grp):
            load_xn(g)

        es_tiles = {}
        sm_state = {}

        def energies(g):
            xt, xd = xt_tiles[g], xd_tiles[g]
            es_g = grp_pool.tile([128, GRP, NJJ], f32, tag="es_g", name=f"es{g}")
            for r in range(GRP):
                pe_ps = ps_e.tile([128, NJJ], f32, tag="pe", name=f"pe{g}_{r}")
                for c in range(NCH):
                    pp = ps_pre.tile([128, TCH], f32, tag="pp", name=f"pp{g}_{r}_{c}")
                    o0 = r * T + c * TCH
                    nc.tensor.matmul(
                        pp[:], lhsT=whT_sb[:], rhs=xt[:, o0:o0 + TCH],
                        start=True, stop=False,
                    )
                    nc.tensor.matmul(
                        pp[:], lhsT=wdT_sb[:], rhs=xd[:, o0:o0 + TCH],
                        start=False, stop=True,
                    )
                    zz = z_pool.tile([128, TCH], bf16, tag="zz", name=f"zz{g}_{r}_{c}")
                    nc.scalar.activation(zz[:], pp[:], AF.Tanh)
                    for q in range(TCH // 128):
                        jj = c * (TCH // 128) + q
                        nc.tensor.matmul(
                            pe_ps[:, jj:jj + 1],
                            lhsT=zz[:, q * 128:(q + 1) * 128],
                            rhs=wa_sb[:],
                            start=True, stop=True,
                        )
                nc.vector.tensor_copy(es_g[:, r, :], pe_ps[:])
            es_tiles[g] = es_g

        def softmax(g):
            lo = g * GRP
            es_g = es_tiles[g]
            esb = grp_pool.tile([128, GRP, NJJ], f32, tag="esb", name=f"esb{g}")
            nc.vector.scalar_tensor_tensor(
                out=esb[:], in0=es_g[:], scalar=ba_sb[:],
                in1=sc_sb[:, lo:lo + GRP, :], op0=OP.add, op1=OP.mult,
            )
            ex_g = grp_pool.tile([128, GRP, NJJ], f32, tag="ex_g", name=f"ex{g}")
            nc.scalar.activation(ex_g[:], esb[:], AF.Exp)
            eb_g = grp_pool.tile([128, GRP, NJJ], bf16, tag="eb_g", name=f"eb{g}")
            nc.vector.tensor_copy(eb_g[:], ex_g[:])
            sums = sm_pool.tile([128, GRP], f32, tag="sums", name=f"sums{g}")
            nc.vector.tensor_reduce(
                out=sums[:], in_=ex_g[:], axis=mybir.AxisListType.X, op=OP.add,
            )
            ptot = ps_sm.tile([GRP, 1], f32, tag="pssm", name=f"ptot{g}")
            nc.tensor.matmul(ptot[:], lhsT=sums[:], rhs=onesc_sb[:], start=True, stop=True)
            tot_sb = sm_pool.tile([GRP, 1], f32, tag="tot", name=f"tot{g}")
            nc.vector.tensor_copy(tot_sb[:], ptot[:])
            ptotr = ps_sm.tile([1, GRP], f32, tag="pssm", name=f"ptotr{g}")
            nc.tensor.transpose(ptotr[:], tot_sb[:], ident_sb[:GRP, :GRP])
            totr_sb = sm_pool.tile([1, GRP], f32, tag="totr", name=f"totr{g}")
            nc.vector.tensor_copy(totr_sb[:], ptotr[:])
            pbc = ps_sm.tile([128, GRP], f32, tag="pssm", name=f"pbc{g}")
            nc.tensor.matmul(pbc[:], lhsT=onesr_sb[:], rhs=totr_sb[:], start=True, stop=True)
            rt = sm_pool.tile([128, GRP], f32, tag="rt", name=f"rt{g}")
            nc.vector.reciprocal(rt[:], pbc[:])
            al_g = grp_pool.tile([128, GRP, NJJ], f32, tag="al_g", name=f"al{g}")
            for r in range(GRP):
                nc.vector.tensor_scalar(
                    out=al_g[:, r, :], in0=ex_g[:, r, :],
                    scalar1=rt[:, r:r + 1], scalar2=None, op0=OP.mult,
                )
            sm_state[g] = (al_g, eb_g, rt)

        ctx_state = {}

        def ctx_mms(g):
            lo = g * GRP
            al_g, eb_g, rt = sm_state[g]
            # unnormalized context (exp weights); one psum bank, one col/sample
            pctx = ps_ctx.tile([128, GRP], f32, tag="pctx", name=f"pctx{g}")
            for r in range(GRP):
                b = lo + r
                for jj in range(NJJ):
                    nc.tensor.matmul(
                        pctx[:, r:r + 1],
                        lhsT=xn_sb[:, b * T + jj * 128:b * T + (jj + 1) * 128],
                        rhs=eb_g[:, r, jj:jj + 1],
                        start=(jj == 0), stop=(jj == NJJ - 1),
                    )
            ctxu = sm_pool.tile([128, GRP], f32, tag="ctxu", name=f"ctxu{g}")
            nc.vector.tensor_copy(ctxu[:], pctx[:])
            ctx_state[g] = ctxu

        def alpha_out_g(g):
            lo = g * GRP
            al_g, eb_g, rt = sm_state[g]
            pat = ps_sm.tile([GRP * NJJ, 128], f32, tag="pssm", name=f"pat{g}")
            nc.tensor.transpose(
                pat[:], al_g[:].rearrange("p r j -> p (r j)"), ident_sb[:]
            )
            at = sm_pool.tile([GRP * NJJ, 128], f32, tag="at", name=f"at{g}")
            nc.vector.tensor_copy(at[:], pat[:])
            nc.scalar.dma_start(
                alpha_out[lo:lo + GRP].rearrange("b (j u) -> (b j) u", j=NJJ), at[:]
            )

        def ctx_out_g(g):
            lo = g * GRP
            al_g, eb_g, rt = sm_state[g]
            ctxu = ctx_state[g]
            ctxn = sm_pool.tile([128, GRP], f32, tag="ctxn", name=f"ctxn{g}")
            nc.vector.tensor_tensor(ctxn[:], ctxu[:], rt[:], op=OP.mult)
            pcg = ps_sm.tile([GRP, 128], f32, tag="pssm", name=f"pcg{g}")
            nc.tensor.transpose(pcg[:], ctxn[:], ident_sb[:])
            cg = sm_pool.tile([GRP, 128], f32, tag="cg", name=f"cg{g}")
            nc.vector.tensor_copy(cg[:], pcg[:])
            nc.scalar.dma_start(ctx_out[lo:lo + GRP], cg[:])

        # ---- software pipeline: E(g), SM(g-1), ALPHA_OUT(g-2); ctx at end ----
        for g in range(n_grp + 2):
            if g < n_grp:
                energies(g)
            if 0 <= g - 1 < n_grp:
                softmax(g - 1)
            if 0 <= g - 2 < n_grp:
                alpha_out_g(g - 2)
        for g in range(n_grp):
            ctx_mms(g)
            ctx_out_g(g)

    nc.compile()
    return nc


def _get_nc():
    if "nc" not in _cache:
        _cache["nc"] = _build_bass()
    return _cache["nc"]


def _prep_core_inputs(Hp_bf, Hd_bf, scale, b_a):
    """Build the per-core input maps (host-side layout transforms)."""
    ident = np.eye(128, dtype=np.float32)
    ones_col = np.ones((128, 1), np.float32)
    ones_row = np.ones((1, 128), np.float32)
    ba_col = np.full((128, 1), np.float32(b_a), np.float32)
    in_maps = []
    for core in range(N_CORES):
        sl = slice(core * B_LOC, (core + 1) * B_LOC)
        hp = Hp_bf[sl]                       # [16, T, H] bf16
        hd = Hd_bf[sl]
        n_grp = B_LOC // GRP
        # [(g r), t, h] -> [g, h, (r t)]
        xposT_p = np.ascontiguousarray(
            hp.reshape(n_grp, GRP, T, H).transpose(0, 3, 1, 2)
        ).reshape(n_grp, H, GRP * T)
        xdynT_p = np.ascontiguousarray(
            hd.reshape(n_grp, GRP, T, H).transpose(0, 3, 1, 2)
        ).reshape(n_grp, H, GRP * T)
        # [(g r), (j p), h] -> [g, p, (r j h)]
        xposN_p = np.ascontiguousarray(
            hp.reshape(n_grp, GRP, NJJ, 128, H).transpose(0, 3, 1, 2, 4)
        ).reshape(n_grp, 128, GRP * T)
        in_maps.append({
            "xposT": xposT_p,
            "xdynT": xdynT_p,
            "xposN": xposN_p,
            # [b, (j p)] -> [p, b, j]
            "scale_all": np.ascontiguousarray(
                scale[sl].reshape(B_LOC, NJJ, 128).transpose(2, 0, 1)
            ),
            "whT": _cache["whT"],
            "wdT": _cache["wdT"],
            "wa": _cache["wa"],
            "ident": ident,
            "ones_col": ones_col,
            "ones_row": ones_row,
            "ba_col": ba_col,
        })
    return in_maps


def kernel(H_pos, H_dyn, acc_w, W_h, W_d, W_a, b_a, beta):
    from concourse.bass_utils import run_bass_kernel_spmd

    H_pos = np.asarray(H_pos, dtype=np.float32)
    H_dyn = np.asarray(H_dyn, dtype=np.float32)
    acc_w = np.asarray(acc_w, dtype=np.float32)
    W_h = np.asarray(W_h, dtype=np.float32)
    W_d = np.asarray(W_d, dtype=np.float32)
    W_a = np.asarray(W_a, dtype=np.float32)
    b_a_f = float(np.asarray(b_a))
    beta_f = float(np.asarray(beta))

    # host scalar/row prep (tiny): softplus(beta), acc normalization, scale
    beta_pos = float(np.log1p(np.exp(beta_f)))
    acc_norm = acc_w / np.clip(acc_w.max(axis=1, keepdims=True), 1e-6, None)
    scale = (1.0 + beta_pos * acc_norm).astype(np.float32)          # [B, T]

    Hp_bf = H_pos.astype(_BF16)
    Hd_bf = H_dyn.astype(_BF16)
    _cache["whT"] = np.ascontiguousarray(W_h.T).astype(_BF16)
    _cache["wdT"] = np.ascontiguousarray(W_d.T).astype(_BF16)
    _cache["wa"] = W_a.reshape(H, 1).astype(_BF16)

    nc = _get_nc()
    in_maps = _prep_core_inputs(Hp_bf, Hd_bf, scale, b_a_f)
    res = run_bass_kernel_spmd(nc, in_maps, list(range(N_CORES)))
    _cache["last_res"] = res

    alpha = np.concatenate([r["alpha_out"] for r in res.results], axis=0)
    context = np.concatenate([r["ctx_out"] for r in res.results], axis=0)
    return (
        context.astype(np.float32, copy=False),
        alpha.astype(np.float32, copy=False),
    )


# revision 20
# speedup vs baseline: 1.1938x; 1.1938x over previous
"""Additive-attention (tanh energy + softmax + context) kernel for 8 TRN2 NeuronCores.

Data-parallel over the batch dim B=128 -> 16 samples per core.

Software-pipelined per-core schedule over groups of 4 samples:
    E(g):   pre.T = W_h @ XposT + W_d @ XdynT (PE) -> tanh (ACT) -> energy
            columns (PE) -> es_g (DVE)
    SM(g):  batched group softmax: (e+b_a)*scale (DVE) -> exp (ACT) -> sums
            (DVE) -> totals/broadcast (PE ones-matmuls) -> 1/total, alpha (DVE)
    A(g):   alpha.T out (one PE transpose + one DMA per group)
    CTX(g): unnormalized ctx.T[:, b] += Xnat_chunk.T @ exp_col (PE, psum),
            then * (1/total), transpose, DMA out
  emitted as E(0), E(1), SM(0), E(2), SM(1), A(0), ..., then all CTX at the end,
  so the PE stream never blocks on the softmax DVE/ACT chain or on the
  (deprioritized) Xnat loads; xt/xd loads carry the phase-1 critical path.

Host side: shard B across 8 cores, cast X to bf16, pre-transpose layouts so every
device DMA is fully contiguous, run SPMD, gather outputs.
"""

import numpy as np
import ml_dtypes

B, T, H = 128, 2048, 128
N_CORES = 8
B_LOC = B // N_CORES      # 16 samples per core
TCH = 512                 # t-chunk for the pre matmuls (one psum bank)
NCH = T // TCH            # 4 chunks per sample
NJJ = T // 128            # 16 energy columns per sample
GRP = 4                   # samples per softmax/ctx group (= DMA batch)

_BF16 = ml_dtypes.bfloat16

_cache = {}


def _build_bass(n_samples=B_LOC):
    import concourse.bass as bass
    import concourse.tile as tile
    from concourse import bacc, mybir
    from contextlib import ExitStack

    f32 = mybir.dt.float32
    bf16 = mybir.dt.bfloat16
    AF = mybir.ActivationFunctionType
    OP = mybir.AluOpType

    assert n_samples % GRP == 0
    n_grp = n_samples // GRP

    nc = bacc.Bacc(
        "TRN2", target_bir_lowering=False, debug=False, num_devices=N_CORES
    )

    xposT = nc.dram_tensor("xposT", [n_grp, H, GRP * T], bf16, kind="ExternalInput").ap()
    xdynT = nc.dram_tensor("xdynT", [n_grp, H, GRP * T], bf16, kind="ExternalInput").ap()
    xposN = nc.dram_tensor("xposN", [n_grp, 128, GRP * T], bf16, kind="ExternalInput").ap()
    # scale_all[p, b, j] = (1 + softplus(beta) * acc_norm)[b, 128*j + p]
    scale_all = nc.dram_tensor("scale_all", [128, B_LOC, NJJ], f32, kind="ExternalInput").ap()
    whT = nc.dram_tensor("whT", [H, H], bf16, kind="ExternalInput").ap()
    wdT = nc.dram_tensor("wdT", [H, H], bf16, kind="ExternalInput").ap()
    wa = nc.dram_tensor("wa", [H, 1], bf16, kind="ExternalInput").ap()
    ident = nc.dram_tensor("ident", [128, 128], f32, kind="ExternalInput").ap()
    ones_col = nc.dram_tensor("ones_col", [128, 1], f32, kind="ExternalInput").ap()
    ones_row = nc.dram_tensor("ones_row", [1, 128], f32, kind="ExternalInput").ap()
    ba_col = nc.dram_tensor("ba_col", [128, 1], f32, kind="ExternalInput").ap()

    alpha_out = nc.dram_tensor("alpha_out", [B_LOC, T], f32, kind="ExternalOutput").ap()
    ctx_out = nc.dram_tensor("ctx_out", [B_LOC, H], f32, kind="ExternalOutput").ap()

    with tile.TileContext(nc) as tc, ExitStack() as ctx:
        consts = ctx.enter_context(tc.tile_pool(name="consts", bufs=1))
        xio_pool = ctx.enter_context(tc.tile_pool(name="xio", bufs=3))
        xn_pool = ctx.enter_context(tc.tile_pool(name="xnp", bufs=1))
        z_pool = ctx.enter_context(tc.tile_pool(name="z", bufs=4))
        sm_pool = ctx.enter_context(tc.tile_pool(name="sm", bufs=5))
        grp_pool = ctx.enter_context(tc.tile_pool(name="grp", bufs=4))
        ps_pre = ctx.enter_context(tc.tile_pool(name="pspre", bufs=3, space="PSUM"))
        ps_e = ctx.enter_context(tc.tile_pool(name="pse", bufs=2, space="PSUM"))
        ps_sm = ctx.enter_context(tc.tile_pool(name="pssm", bufs=2, space="PSUM"))
        ps_ctx = ctx.enter_context(tc.tile_pool(name="psctx", bufs=1, space="PSUM"))

        # constants on the scalar (ACT) HWDGE ring; bulk X loads on the sync ring
        whT_sb = consts.tile([H, H], bf16, tag="whT")
        nc.scalar.dma_start(whT_sb[:], whT[:])
        wdT_sb = consts.tile([H, H], bf16, tag="wdT")
        nc.scalar.dma_start(wdT_sb[:], wdT[:])
        wa_sb = consts.tile([H, 1], bf16, tag="wa")
        nc.scalar.dma_start(wa_sb[:], wa[:])
        ident_sb = consts.tile([128, 128], f32, tag="ident")
        nc.scalar.dma_start(ident_sb[:], ident[:])
        onesc_sb = consts.tile([128, 1], f32, tag="onesc")
        nc.scalar.dma_start(onesc_sb[:], ones_col[:])
        onesr_sb = consts.tile([1, 128], f32, tag="onesr")
        nc.scalar.dma_start(onesr_sb[:], ones_row[:])
        ba_sb = consts.tile([128, 1], f32, tag="ba")
        nc.scalar.dma_start(ba_sb[:], ba_col[:])
        sc_sb = consts.tile([128, B_LOC, NJJ], f32, tag="sc")
        nc.scalar.dma_start(sc_sb[:], scale_all[:])

        xn_sb = xn_pool.tile([128, B_LOC * T], bf16, tag="xn")

        # ---- input DMAs on the sync ring, phase-1-critical first ----
        xt_tiles = {}
        xd_tiles = {}

        def load_group(g):
            xt = xio_pool.tile([H, GRP * T], bf16, tag="xt", name=f"xt{g}")
            xd = xio_pool.tile([H, GRP * T], bf16, tag="xd", name=f"xd{g}")
            if g == 0:
                # split first group for a fast pipeline start
                hf = GRP * T // 2
                nc.sync.dma_start(xt[:, :hf], xposT[g][:, :hf])
                nc.sync.dma_start(xd[:, :hf], xdynT[g][:, :hf])
                nc.sync.dma_start(xt[:, hf:], xposT[g][:, hf:])
                nc.sync.dma_start(xd[:, hf:], xdynT[g][:, hf:])
            else:
                nc.sync.dma_start(xt[:], xposT[g])
                nc.sync.dma_start(xd[:], xdynT[g])
            xt_tiles[g] = xt
            xd_tiles[g] = xd

        def load_xn(g):
            nc.sync.dma_start(
                xn_sb[:, g * GRP * T:(g + 1) * GRP * T], xposN[g]
            )

        for g in range(n_grp):
            load_group(g)
        for g in range(n_grp):
            load_xn(g)

        es_tiles = {}
        sm_state = {}

        def energies(g):
            xt, xd = xt_tiles[g], xd_tiles[g]
            es_g = grp_pool.tile([128, GRP, NJJ], f32, tag="es_g", name=f"es{g}")
            for r in range(GRP):
                pe_ps = ps_e.tile([128, NJJ], f32, tag="pe", name=f"pe{g}_{r}")
                for c in range(NCH):
                    pp = ps_pre.tile([128, TCH], f32, tag="pp", name=f"pp{g}_{r}_{c}")
                    o0 = r * T + c * TCH
                    nc.tensor.matmul(
                        pp[:], lhsT=whT_sb[:], rhs=xt[:, o0:o0 + TCH],
                        start=True, stop=False,
                    )
                    nc.tensor.matmul(
                        pp[:], lhsT=wdT_sb[:], rhs=xd[:, o0:o0 + TCH],
                        start=False, stop=True,
                    )
                    zz = z_pool.tile([128, TCH], bf16, tag="zz", name=f"zz{g}_{r}_{c}")
                    nc.scalar.activation(zz[:], pp[:], AF.Tanh)
                    for q in range(TCH // 128):
                        jj = c * (TCH // 128) + q
                        nc.tensor.matmul(
                            pe_ps[:, jj:jj + 1],
                            lhsT=zz[:, q * 128:(q + 1) * 128],
                            rhs=wa_sb[:],
                            start=True, stop=True,
                        )
                nc.vector.tensor_copy(es_g[:, r, :], pe_ps[:])
            es_tiles[g] = es_g

        def softmax(g):
            lo = g * GRP
            es_g = es_tiles[g]
            esb = grp_pool.tile([128, GRP, NJJ], f32, tag="esb", name=f"esb{g}")
            nc.vector.scalar_tensor_tensor(
                out=esb[:], in0=es_g[:], scalar=ba_sb[:],
                in1=sc_sb[:, lo:lo + GRP, :], op0=OP.add, op1=OP.mult,
            )
            ex_g = grp_pool.tile([128, GRP, NJJ], f32, tag="ex_g", name=f"ex{g}")
            nc.scalar.activation(ex_g[:], esb[:], AF.Exp)
            eb_g = grp_pool.tile([128, GRP, NJJ], bf16, tag="eb_g", name=f"eb{g}")
            nc.vector.tensor_copy(eb_g[:], ex_g[:])
            sums = sm_pool.tile([128, GRP], f32, tag="sums", name=f"sums{g}")
            nc.vector.tensor_reduce(
                out=sums[:], in_=ex_g[:], axis=mybir.AxisListType.X, op=OP.add,
            )
            ptot = ps_sm.tile([GRP, 1], f32, tag="pssm", name=f"ptot{g}")
            nc.tensor.matmul(ptot[:], lhsT=sums[:], rhs=onesc_sb[:], start=True, stop=True)
            tot_sb = sm_pool.tile([GRP, 1], f32, tag="tot", name=f"tot{g}")
            nc.vector.tensor_copy(tot_sb[:], ptot[:])
            ptotr = ps_sm.tile([1, GRP], f32, tag="pssm", name=f"ptotr{g}")
            nc.tensor.transpose(ptotr[:], tot_sb[:], ident_sb[:GRP, :GRP])
            totr_sb = sm_pool.tile([1, GRP], f32, tag="totr", name=f"totr{g}")
            nc.vector.tensor_copy(totr_sb[:], ptotr[:])
            pbc = ps_sm.tile([128, GRP], f32, tag="pssm", name=f"pbc{g}")
            nc.tensor.matmul(pbc[:], lhsT=onesr_sb[:], rhs=totr_sb[:], start=True, stop=True)
            rt = sm_pool.tile([128, GRP], f32, tag="rt", name=f"rt{g}")
            nc.vector.reciprocal(rt[:], pbc[:])
            al_g = grp_pool.tile([128, GRP, NJJ], f32, tag="al_g", name=f"al{g}")
            for r in range(GRP):
                nc.vector.tensor_scalar(
                    out=al_g[:, r, :], in0=ex_g[:, r, :],
                    scalar1=rt[:, r:r + 1], scalar2=None, op0=OP.mult,
                )
            sm_state[g] = (al_g, eb_g, rt)

        ctx_state = {}

        def ctx_mms(g):
            lo = g * GRP
            al_g, eb_g, rt = sm_state[g]
            # unnormalized context (exp weights); one psum bank, one col/sample
            pctx = ps_ctx.tile([128, GRP], f32, tag="pctx", name=f"pctx{g}")
            for r in range(GRP):
                b = lo + r
                for jj in range(NJJ):
                    nc.tensor.matmul(
                        pctx[:, r:r + 1],
                        lhsT=xn_sb[:, b * T + jj * 128:b * T + (jj + 1) * 128],
                        rhs=eb_g[:, r, jj:jj + 1],
                        start=(jj == 0), stop=(jj == NJJ - 1),
                    )
            ctxu = sm_pool.tile([128, GRP], f32, tag="ctxu", name=f"ctxu{g}")
            nc.vector.tensor_copy(ctxu[:], pctx[:])
            ctx_state[g] = ctxu

        def alpha_out_g(g):
            lo = g * GRP
            al_g, eb_g, rt = sm_state[g]
            pat = ps_sm.tile([GRP * NJJ, 128], f32, tag="pssm", name=f"pat{g}")
            nc.tensor.transpose(
                pat[:], al_g[:].rearrange("p r j -> p (r j)"), ident_sb[:]
            )
            at = sm_pool.tile([GRP * NJJ, 128], f32, tag="at", name=f"at{g}")
            nc.vector.tensor_copy(at[:], pat[:])
            nc.scalar.dma_start(
                alpha_out[lo:lo + GRP].rearrange("b (j u) -> (b j) u", j=NJJ), at[:]
            )

        def ctx_out_g(g):
            lo = g * GRP
            al_g, eb_g, rt = sm_state[g]
            ctxu = ctx_state[g]
            ctxn = sm_pool.tile([128, GRP], f32, tag="ctxn", name=f"ctxn{g}")
            nc.vector.tensor_tensor(ctxn[:], ctxu[:], rt[:], op=OP.mult)
            pcg = ps_sm.tile([GRP, 128], f32, tag="pssm", name=f"pcg{g}")
            nc.tensor.transpose(pcg[:], ctxn[:], ident_sb[:])
            cg = sm_pool.tile([GRP, 128], f32, tag="cg", name=f"cg{g}")
            nc.vector.tensor_copy(cg[:], pcg[:])
            nc.scalar.dma_start(ctx_out[lo:lo + GRP], cg[:])

        # ---- software pipeline: E(g), SM(g-1), ALPHA_OUT(g-2); ctx at end ----
        for g in range(n_grp + 2):
            if g < n_grp:
                energies(g)
            if 0 <= g - 1 < n_grp:
                softmax(g - 1)
            if 0 <= g - 2 < n_grp:
                alpha_out_g(g - 2)
        for g in range(n_grp):
            ctx_mms(g)
            ctx_out_g(g)

    nc.compile()
    return nc


def _get_nc():
    if "nc" not in _cache:
        _cache["nc"] = _build_bass()
    return _cache["nc"]


def _prep_core_inputs(Hp_bf, Hd_bf, scale, b_a):
    """Build the per-core input maps (host-side layout transforms)."""
    ident = np.eye(128, dtype=np.float32)
    ones_col = np.ones((128, 1), np.float32)
    ones_row = np.ones((1, 128), np.float32)
    ba_col = np.full((128, 1), np.float32(b_a), np.float32)
    in_maps = []
    for core in range(N_CORES):
        sl = slice(core * B_LOC, (core + 1) * B_LOC)
        hp = Hp_bf[sl]                       # [16, T, H] bf16
        hd = Hd_bf[sl]
        n_grp = B_LOC // GRP
        # [(g r), t, h] -> [g, h, (r t)]
        xposT_p = np.ascontiguousarray(
            hp.reshape(n_grp, GRP, T, H).transpose(0, 3, 1, 2)
        ).reshape(n_grp, H, GRP * T)
        xdynT_p = np.ascontiguousarray(
            hd.reshape(n_grp, GRP, T, H).transpose(0, 3, 1, 2)
        ).reshape(n_grp, H, GRP * T)
        # [(g r), (j p), h] -> [g, p, (r j h)]
        xposN_p = np.ascontiguousarray(
            hp.reshape(n_grp, GRP, NJJ, 128, H).transpose(0, 3, 1, 2, 4)
        ).reshape(n_grp, 128, GRP * T)
        in_maps.append({
            "xposT": xposT_p,
            "xdynT": xdynT_p,
            "xposN": xposN_p,
            # [b, (j p)] -> [p, b, j]
            "scale_all": np.ascontiguousarray(
                scale[sl].reshape(B_LOC, NJJ, 128).transpose(2, 0, 1)
            ),
            "whT": _cache["whT"],
            "wdT": _cache["wdT"],
            "wa": _cache["wa"],
            "ident": ident,
            "ones_col": ones_col,
            "ones_row": ones_row,
            "ba_col": ba_col,
        })
    return in_maps


def kernel(H_pos, H_dyn, acc_w, W_h, W_d, W_a, b_a, beta):
    from concourse.bass_utils import run_bass_kernel_spmd

    H_pos = np.asarray(H_pos, dtype=np.float32)
    H_dyn = np.asarray(H_dyn, dtype=np.float32)
    acc_w = np.asarray(acc_w, dtype=np.float32)
    W_h = np.asarray(W_h, dtype=np.float32)
    W_d = np.asarray(W_d, dtype=np.float32)
    W_a = np.asarray(W_a, dtype=np.float32)
    b_a_f = float(np.asarray(b_a))
    beta_f = float(np.asarray(beta))

    # host scalar/row prep (tiny): softplus(beta), acc normalization, scale
    beta_pos = float(np.log1p(np.exp(beta_f)))
    acc_norm = acc_w / np.clip(acc_w.max(axis=1, keepdims=True), 1e-6, None)
    scale = (1.0 + beta_pos * acc_norm).astype(np.float32)          # [B, T]

    Hp_bf = H_pos.astype(_BF16)
    Hd_bf = H_dyn.astype(_BF16)
    _cache["whT"] = np.ascontiguousarray(W_h.T).astype(_BF16)
    _cache["wdT"] = np.ascontiguousarray(W_d.T).astype(_BF16)
    _cache["wa"] = W_a.reshape(H, 1).astype(_BF16)

    nc = _get_nc()
    in_maps = _prep_core_inputs(Hp_bf, Hd_bf, scale, b_a_f)
    res = run_bass_kernel_spmd(nc, in_maps, list(range(N_CORES)))
    _cache["last_res"] = res

    alpha = np.concatenate([r["alpha_out"] for r in res.results], axis=0)
    context = np.concatenate([r["ctx_out"] for r in res.results], axis=0)
    return (
        context.astype(np.float32, copy=False),
        alpha.astype(np.float32, copy=False),
    )
